# revision 41
# baseline (speedup 1.0000x reference)
"""Multi-head attention Trainium2 kernel (8 NeuronCores, SPMD), v2.

Problem: B=2, S=2048, D=1024, H=16 heads, DK=DV=64.
Sharding: batch (2) x head-groups (4 heads per core) = 8 cores.
Each core computes, for its batch b and its 4 heads, Q/K/V projections,
attention, and the partial output projection ctx @ Wo[head rows]; the host
sums the 4 partials per batch and adds the constant row bo + concat(bv) @ Wo
(the V bias is a constant shift of ctx because softmax rows sum to 1, and
the K bias drops entirely by softmax shift invariance).

Optimizations vs the 427us fp32r baseline (final: ~226us):
  - All matmuls bf16 (1 cyc/col; LDWEIGHTS fully hidden behind matmuls).
  - x is transposed on the HOST; no on-chip transposes at all.
  - Scores for the two heads of a pair run CONCURRENTLY in the PE array
    (row tiling: K=64 each at tile_position (0,0)/(64,0), emitted
    adjacently into different PSUM banks).
  - exp() alternates engines per key-tile: exact LUT exp on ScalarE,
    Schraudolph bf16-bits exp on VectorE (int16(round(x*184.665+16251.39))
    reinterpreted as bf16; end-to-end rel err ~6.8e-3 vs the 2e-2 gate).
  - Softmax denominators ride the ctx matmul as a ones-column of V (row 64
    of the 65-row PSUM accumulator). 1/den: DMA-bounce through DRAM
    reshapes each [1,1024] den row to [16,64] so the 8-cyc/elem DVE
    reciprocal costs ~0.55us, then a broadcast DMA fans 1/den across
    partitions for one normalize-multiply per pair.
  - PSUM budget (8 banks): 5 x 1-bank score tiles (deep rotation so the
    PE never waits on exp) + 3 ctx accumulator banks. Steady-state
    attention runs the PE at 96-100% occupancy (~873ns per key-tile).
  - bf16 output partials (host sums in fp32); biases: K bias dropped
    (softmax shift invariance), V bias folded into a host-side constant
    row (bo + concat(bv) @ Wo), Q bias applied during PSUM evacuation.
"""
import sys

if "/opt/trn_rl_repo" not in sys.path:
    sys.path.insert(0, "/opt/trn_rl_repo")

import ml_dtypes
import numpy as np

import bass_rust
import concourse.bass as bass
import concourse.mybir as mybir
import concourse.tile as tile
from concourse.bass_utils import run_bass_kernel_spmd
from concourse.vector_clock import ScopedClock

F32 = mybir.dt.float32
BF16 = mybir.dt.bfloat16
I16 = mybir.dt.int16
AF = mybir.ActivationFunctionType
ALU = mybir.AluOpType
NPBF16 = ml_dtypes.bfloat16

B, S, D = 2, 2048, 1024
H, DK, DV = 16, 64, 64
HL = 4          # heads per core
NPAIR = 2       # head pairs per core (2 heads packed per 128 partitions)
ST = S // 128   # 16 s-tiles / t-tiles
DT = D // 128   # 8 d-tiles
SC = 512        # attention s-chunk (one PSUM bank)
NSC = S // SC   # 4
N_CORES = 8

EXP_A = 184.66496   # 2^7 * log2(e)
EXP_B = 16251.39    # 2^7 * (127 - c_minimax)


class _TileContextSplitDrain(tile.TileContext):
    """Walrus in this container rejects ANY instruction carrying >1 sem wait
    ("Too many sync wait commands"). Post-lowering, sweep every basic block
    and move surplus waits onto injected EventSemaphore carrier instructions
    placed immediately before the over-subscribed instruction (same engine,
    same program point - semantics unchanged). Also emit the final drain as
    one drain per logical proc so each carries a single wait."""

    _MAXW = 1

    def _split_excess_waits(self):
        nc = self.nc
        for fn in nc.m.functions:
            for bb in fn.blocks:
                insts = bb.instructions
                new_list = []
                changed = False
                for ins in insts:
                    si = ins.sync_info
                    waits = list(si.on_wait) if si is not None and si.on_wait else []
                    if len(waits) > self._MAXW:
                        changed = True
                        extra, keep = waits[:-self._MAXW], waits[-self._MAXW:]
                        for k in range(0, len(extra), self._MAXW):
                            chunk = extra[k:k + self._MAXW]
                            ev = mybir.InstEventSemaphore(
                                name=f"wsplit_{nc.next_id()}", ins=[], outs=[]
                            )
                            ev.engine = ins.engine
                            ev.sync_info = bass_rust.SyncInfo(
                                on_wait=chunk, on_update=[]
                            )
                            nc.register_instruction(ev, overwrite=True)
                            new_list.append(ev)
                        ins.sync_info = bass_rust.SyncInfo(
                            on_wait=keep,
                            on_update=list(si.on_update) if si.on_update else [],
                        )
                    new_list.append(ins)
                if changed:
                    insts[:] = new_list

    def _drain_and_barrier(self, tick_clock, wait_clock):
        self._split_excess_waits()
        ticks = list(tick_clock.global_clock)
        for p, t in enumerate(ticks):
            if t <= 0:
                continue
            v = bass_rust.VectorClock()
            v.require_at_least(p, t)
            d = self.nc.sync.drain()
            wait_clock.add_sem_waits(d.ins, ScopedClock({None: v}))
        self.nc.all_engine_barrier()
        popped = self.nc._tile_sem_poison_stack.pop()
        assert popped is self._sem_poison
        self.nc.clear_and_free_semaphores(list(self.sems.allocated().values()))
        self.nc.all_engine_barrier()


def build_nc() -> bass.Bass:
    nc = bass.Bass()

    # host-pretiled inputs
    xt_d = nc.dram_tensor("xt", [128, DT, S], BF16, kind="ExternalInput")
    wqkv_d = nc.dram_tensor("wqkv", [128, DT, 6 * 128], BF16, kind="ExternalInput")
    wo_d = nc.dram_tensor("wo", [128, NPAIR, D], BF16, kind="ExternalInput")
    bq_d = nc.dram_tensor("bq", [128, NPAIR], F32, kind="ExternalInput")
    out_d = nc.dram_tensor("out", [S, D], BF16, kind="ExternalOutput")

    with _TileContextSplitDrain(nc) as tc:
        with (
            tc.tile_pool(name="const", bufs=1) as constp,
            tc.tile_pool(name="pers", bufs=1) as pers,
            tc.tile_pool(name="dramsc", bufs=1, space="DRAM") as drp,
        ):
            bq_sb = constp.tile([128, NPAIR], F32, tag="bq")
            nc.sync.dma_start(out=bq_sb, in_=bq_d[:, :])
            wo_sb = constp.tile([128, NPAIR, D], BF16, tag="wo")
            nc.sync.dma_start(out=wo_sb, in_=wo_d[:, :, :])
            # per-d-tile DMAs so the first QKV matmuls start ~3us in, not
            # after the whole 5.5MB input load
            xT = pers.tile([128, DT, S], BF16, tag="xT")
            wq_sb = pers.tile([128, DT, 6 * 128], BF16, tag="wqkv")
            for d in range(DT):
                nc.sync.dma_start(out=wq_sb[:, d, :], in_=wqkv_d[:, d, :])
                nc.sync.dma_start(out=xT[:, d, :], in_=xt_d[:, d, :])

            QT = pers.tile([128, NPAIR, S], BF16, tag="QT")
            KT = pers.tile([128, NPAIR, S], BF16, tag="KT")
            ctxT = pers.tile([128, NPAIR, S], BF16, tag="ctxT")
            ctxR = pers.tile([128, NPAIR, S], BF16, tag="ctxR")
            V_aug = pers.tile([128, ST, HL, 66], BF16, tag="V_aug")
            den_d = drp.tile([2 * NSC, 2 * SC], F32, tag="den_d")
            recip_d = drp.tile([2 * NSC, 16, 64], F32, tag="recip_d")

            # ---------------- Phase 1: Q^T / K^T projections -----------------
            # 2-bank half-groups with 4-deep rotation keep the PE stream
            # dense across the evac boundaries; evacs alternate engines
            with tc.tile_pool(name="qkps", bufs=4, space="PSUM") as qkp:
                for gi, (proj, pair, ch) in enumerate(
                    (p, q, h) for p in range(2) for q in range(NPAIR)
                    for h in range(2)
                ):
                    ps = qkp.tile([128, 2, 512], F32, tag="qkps")
                    col = (proj * 2 + pair) * 128
                    for d in range(DT):
                        lhs = wq_sb[:, d, col:col + 128]
                        for c2 in range(2):
                            c4 = 2 * ch + c2
                            nc.tensor.matmul(
                                ps[:, c2, :],
                                lhs,
                                xT[:, d, c4 * 512:(c4 + 1) * 512],
                                start=(d == 0),
                                stop=(d == DT - 1),
                            )
                    for c2 in range(2):
                        c4 = 2 * ch + c2
                        dst = (QT if proj == 0 else KT)[
                            :, pair, c4 * 512:(c4 + 1) * 512
                        ]
                        if proj == 0:
                            if (gi + c2) % 2 == 0:
                                nc.vector.tensor_scalar_add(
                                    dst, ps[:, c2, :], bq_sb[:, pair:pair + 1]
                                )
                            else:
                                nc.scalar.activation(
                                    dst, ps[:, c2, :], AF.Identity,
                                    bias=bq_sb[:, pair:pair + 1],
                                )
                        else:
                            if (gi + c2) % 2 == 0:
                                nc.scalar.activation(dst, ps[:, c2, :], AF.Copy)
                            else:
                                nc.vector.tensor_copy(dst, ps[:, c2, :])

            # ---------------- Phase 2: V (direct layout) + ones column -------
            nc.vector.memset(V_aug[:, :, :, 64:65], 1.0)
            nc.vector.memset(V_aug[:, :, :, 65:66], 0.0)
            with tc.tile_pool(name="vps", bufs=3, space="PSUM") as vp:
                for t in range(ST):
                    ps = vp.tile([128, HL * DV], F32, tag="vps")
                    for d in range(DT):
                        nc.tensor.matmul(
                            ps,
                            xT[:, d, t * 128:(t + 1) * 128],
                            wq_sb[:, d, 512:768],
                            start=(d == 0),
                            stop=(d == DT - 1),
                        )
                    nc.scalar.activation(
                        V_aug[:, t, :, 0:64],
                        ps.rearrange("p (h v) -> p h v", h=HL),
                        AF.Copy,
                    )

            # ---------------- Phase 3: attention -----------------------------
            with (
                tc.tile_pool(name="ptp", bufs=4) as ptp,
                tc.tile_pool(name="repp", bufs=4) as repp,
                tc.tile_pool(name="denp", bufs=2) as denp,
                tc.tile_pool(name="outp", bufs=6) as outp,
            ):
                def norm_pair(c, pair, den_g):
                    """Normalize this pair's chunk: 1/den via a tall-skinny
                    [16,64] reshape (DMA reshuffle makes the 8-cyc/elem DVE
                    reciprocal ~0.55us), broadcast, scale."""
                    rp = 2 * c + pair
                    den_sb = denp.tile([16, 64], F32, name="densb", tag="densb")
                    recip_sb = denp.tile([16, 64], F32, name="recsb", tag="recsb")
                    nc.sync.dma_start(out=den_d[rp:rp + 1, :], in_=den_g)
                    nc.sync.dma_start(
                        out=den_sb,
                        in_=den_d[rp, :].unsqueeze(0)
                        .rearrange("q (p k) -> (q p) k", p=16),
                    )
                    nc.vector.reciprocal(recip_sb, den_sb)
                    nc.sync.dma_start(out=recip_d[rp, :, :], in_=recip_sb)
                    rep = repp.tile([128, SC], F32, tag="rep")
                    for e in range(2):
                        nc.sync.dma_start(
                            out=rep[64 * e:64 * (e + 1), :],
                            in_=recip_d[rp, 8 * e:8 * (e + 1), :]
                            .rearrange("p k -> (p k)").unsqueeze(0)
                            .to_broadcast((64, SC)),
                        )
                    nc.vector.tensor_mul(
                        ctxT[:, pair, c * SC:(c + 1) * SC],
                        ctxR[:, pair, c * SC:(c + 1) * SC],
                        rep,
                    )

                def outproj_tile(i, ops_pool):
                    ps = ops_pool.tile([128, 2, 512], F32, name="ops", tag="ops")
                    for dc in range(2):
                        for pair in range(NPAIR):
                            nc.tensor.matmul(
                                ps[:, dc, :],
                                ctxT[:, pair, i * 128:(i + 1) * 128],
                                wo_sb[:, pair, dc * 512:(dc + 1) * 512],
                                start=(pair == 0),
                                stop=(pair == NPAIR - 1),
                            )
                    # whole-tile evac on one engine, alternating per tile:
                    # halves the cross-engine handoffs in the drain
                    ot = outp.tile([128, D], BF16, tag="ot")
                    if i % 2 == 0:
                        nc.scalar.activation(ot, ps.rearrange("p a b -> p (a b)"),
                                             AF.Copy)
                    else:
                        nc.vector.tensor_copy(ot, ps.rearrange("p a b -> p (a b)"))
                    nc.sync.dma_start(
                        out=out_d[i * 128:(i + 1) * 128, :], in_=ot
                    )

                def attn_chunk(c, spp, cpp):
                    """Both pairs' j-loops interleaved: pair0's exp runs on
                    ScalarE (exact), pair1's on VectorE (Schraudolph), so the
                    two exp streams and the pair-end evac/norm chains hide
                    inside each other's PE stream."""
                    cps = [
                        [
                            cpp.tile([65, SC], F32, name=f"cp{p}{e}", tag="cp")
                            for e in range(2)
                        ]
                        for p in range(NPAIR)
                    ]
                    for j in range(ST):
                        for pair in range(NPAIR):
                            sps = [
                                spp.tile([128, SC], F32, name=f"sp{e}", tag="sp")
                                for e in range(2)
                            ]
                            for e in range(2):
                                nc.tensor.matmul(
                                    sps[e],
                                    KT[64 * e:64 * (e + 1), pair,
                                       j * 128:(j + 1) * 128],
                                    QT[64 * e:64 * (e + 1), pair,
                                       c * SC:(c + 1) * SC],
                                    start=True,
                                    stop=True,
                                )
                            pt = ptp.tile([128, 2, SC], BF16, tag="pt")
                            for e in range(2):
                                if pair == 0:
                                    nc.scalar.activation(
                                        pt[:, e, :], sps[e], AF.Exp
                                    )
                                else:
                                    nc.vector.tensor_scalar(
                                        pt[:, e, :].bitcast(I16),
                                        sps[e],
                                        EXP_A,
                                        EXP_B,
                                        ALU.mult,
                                        ALU.add,
                                    )
                            for e in range(2):
                                nc.tensor.matmul(
                                    cps[pair][e][0:65, :],
                                    V_aug[:, j, 2 * pair + e, 0:65],
                                    pt[:, e, :],
                                    start=(j == 0),
                                    stop=(j == ST - 1),
                                )
                    for pair in range(NPAIR):
                        # stage raw ctx + denominators, then normalize
                        den_g = denp.tile([1, 2 * SC], F32, name="den", tag="den")
                        for e in range(2):
                            nc.vector.tensor_copy(
                                den_g[0:1, e * SC:(e + 1) * SC],
                                cps[pair][e][64:65, :],
                            )
                        nc.scalar.activation(
                            ctxR[0:64, pair, c * SC:(c + 1) * SC],
                            cps[pair][0][0:64, :],
                            AF.Copy,
                        )
                        nc.vector.tensor_copy(
                            ctxR[64:128, pair, c * SC:(c + 1) * SC],
                            cps[pair][1][0:64, :],
                        )
                        norm_pair(c, pair, den_g)

                with (
                    tc.tile_pool(name="spp", bufs=4, space="PSUM") as spp,
                    tc.tile_pool(name="cpp", bufs=4, space="PSUM") as cpp,
                ):
                    for c in range(NSC):
                        attn_chunk(c, spp, cpp)

                # ------------ Phase 4: output projection ----------------------
                with tc.tile_pool(name="ops2", bufs=3, space="PSUM") as ops2:
                    for i in range(ST):
                        outproj_tile(i, ops2)

    return nc


_NC_CACHE = None


def get_nc() -> bass.Bass:
    global _NC_CACHE
    if _NC_CACHE is None:
        _NC_CACHE = build_nc()
    return _NC_CACHE


def prep_in_maps(hidden_state, Wq, bq, Wk, bk, Wv, bv, Wo, bo):
    hidden_state = np.asarray(hidden_state, np.float32)
    Wq, bq = np.asarray(Wq, np.float32), np.asarray(bq, np.float32)
    Wk = np.asarray(Wk, np.float32)
    Wv = np.asarray(Wv, np.float32)
    Wo = np.asarray(Wo, np.float32)
    scale = np.float32(1.0 / np.sqrt(DK))

    # shared per-batch transposed activations: [128, DT, S] bf16
    xts = []
    for b in range(B):
        xt = np.ascontiguousarray(hidden_state[b].T)          # [D, S]
        xt = xt.reshape(DT, 128, S).transpose(1, 0, 2)        # [128, DT, S]
        xts.append(np.ascontiguousarray(xt.astype(NPBF16)))

    in_maps = []
    for core in range(N_CORES):
        b, g = core // 4, core % 4
        hs = slice(HL * g, HL * (g + 1))
        # [4, D, DK] heads -> pair-major column blocks of 128
        wq = (Wq[hs] * scale).transpose(1, 0, 2).reshape(D, HL * DK)
        wk = Wk[hs].transpose(1, 0, 2).reshape(D, HL * DK)
        wv = Wv[hs].transpose(1, 0, 2).reshape(D, HL * DV)    # head-major cols
        wqkv = np.concatenate([wq, wk, wv], axis=1)           # [D, 768]
        wqkv = wqkv.reshape(DT, 128, 6 * 128).transpose(1, 0, 2)
        wo = Wo[HL * DV * g: HL * DV * (g + 1)]               # [256, D]
        wo = wo.reshape(NPAIR, 128, D).transpose(1, 0, 2)     # [128, 2, D]
        bq_p = (bq[hs] * scale).reshape(NPAIR, 128).T         # [128, 2]
        in_maps.append({
            "xt": xts[b],
            "wqkv": np.ascontiguousarray(wqkv.astype(NPBF16)),
            "wo": np.ascontiguousarray(wo.astype(NPBF16)),
            "bq": np.ascontiguousarray(bq_p.astype(np.float32)),
        })
    return in_maps


def gather(results, bv, Wo, bo):
    """Sum the 4 row-parallel partials per batch + constant bias row."""
    bias = (
        np.asarray(bv, np.float32).reshape(H * DV) @ np.asarray(Wo, np.float32)
        + np.asarray(bo, np.float32)
    )
    out = np.empty((B, S, D), np.float32)
    for b in range(B):
        acc = results[4 * b]["out"].astype(np.float32)
        for g in range(1, 4):
            acc = acc + results[4 * b + g]["out"]
        out[b] = acc + bias
    return out


def kernel(**inputs) -> np.ndarray:
    nc = get_nc()
    in_maps = prep_in_maps(**inputs)
    res = run_bass_kernel_spmd(nc, in_maps, core_ids=list(range(N_CORES)))
    return gather(res.results, inputs["bv"], inputs["Wo"], inputs["bo"])


# revision 43
# speedup vs baseline: 1.0449x; 1.0449x over previous
"""Multi-head attention Trainium2 kernel (8 NeuronCores, SPMD), v2.

Problem: B=2, S=2048, D=1024, H=16 heads, DK=DV=64.
Sharding: batch (2) x head-groups (4 heads per core) = 8 cores.
Each core computes, for its batch b and its 4 heads, Q/K/V projections,
attention, and the partial output projection ctx @ Wo[head rows]; the host
sums the 4 partials per batch and adds the constant row bo + concat(bv) @ Wo
(the V bias is a constant shift of ctx because softmax rows sum to 1, and
the K bias drops entirely by softmax shift invariance).

Optimizations vs the 427us fp32r baseline (final: ~226us):
  - All matmuls bf16 (1 cyc/col; LDWEIGHTS fully hidden behind matmuls).
  - x is transposed on the HOST; no on-chip transposes at all.
  - Scores for the two heads of a pair run CONCURRENTLY in the PE array
    (row tiling: K=64 each at tile_position (0,0)/(64,0), emitted
    adjacently into different PSUM banks).
  - exp() alternates engines per key-tile: exact LUT exp on ScalarE,
    Schraudolph bf16-bits exp on VectorE (int16(round(x*184.665+16251.39))
    reinterpreted as bf16; end-to-end rel err ~6.8e-3 vs the 2e-2 gate).
  - Softmax denominators ride the ctx matmul as a ones-column of V (row 64
    of the 65-row PSUM accumulator). 1/den: DMA-bounce through DRAM
    reshapes each [1,1024] den row to [16,64] so the 8-cyc/elem DVE
    reciprocal costs ~0.55us, then a broadcast DMA fans 1/den across
    partitions for one normalize-multiply per pair.
  - PSUM budget (8 banks): 5 x 1-bank score tiles (deep rotation so the
    PE never waits on exp) + 3 ctx accumulator banks. Steady-state
    attention runs the PE at 96-100% occupancy (~873ns per key-tile).
  - bf16 output partials (host sums in fp32); biases: K bias dropped
    (softmax shift invariance), V bias folded into a host-side constant
    row (bo + concat(bv) @ Wo), Q bias applied during PSUM evacuation.
"""
import sys

if "/opt/trn_rl_repo" not in sys.path:
    sys.path.insert(0, "/opt/trn_rl_repo")

import ml_dtypes
import numpy as np

import bass_rust
import concourse.bass as bass
import concourse.mybir as mybir
import concourse.tile as tile
from concourse.bass_utils import run_bass_kernel_spmd
from concourse.vector_clock import ScopedClock

F32 = mybir.dt.float32
BF16 = mybir.dt.bfloat16
I16 = mybir.dt.int16
AF = mybir.ActivationFunctionType
ALU = mybir.AluOpType
NPBF16 = ml_dtypes.bfloat16

B, S, D = 2, 2048, 1024
H, DK, DV = 16, 64, 64
HL = 4          # heads per core
NPAIR = 2       # head pairs per core (2 heads packed per 128 partitions)
ST = S // 128   # 16 s-tiles / t-tiles
DT = D // 128   # 8 d-tiles
SC = 512        # attention s-chunk (one PSUM bank)
NSC = S // SC   # 4
N_CORES = 8

EXP_A = 184.66496   # 2^7 * log2(e)
EXP_B = 16251.39    # 2^7 * (127 - c_minimax)


class _TileContextSplitDrain(tile.TileContext):
    """Walrus in this container rejects ANY instruction carrying >1 sem wait
    ("Too many sync wait commands"). Post-lowering, sweep every basic block
    and move surplus waits onto injected EventSemaphore carrier instructions
    placed immediately before the over-subscribed instruction (same engine,
    same program point - semantics unchanged). Also emit the final drain as
    one drain per logical proc so each carries a single wait."""

    _MAXW = 1

    def _split_excess_waits(self):
        nc = self.nc
        for fn in nc.m.functions:
            for bb in fn.blocks:
                insts = bb.instructions
                new_list = []
                changed = False
                for ins in insts:
                    si = ins.sync_info
                    waits = list(si.on_wait) if si is not None and si.on_wait else []
                    if len(waits) > self._MAXW:
                        changed = True
                        extra, keep = waits[:-self._MAXW], waits[-self._MAXW:]
                        for k in range(0, len(extra), self._MAXW):
                            chunk = extra[k:k + self._MAXW]
                            ev = mybir.InstEventSemaphore(
                                name=f"wsplit_{nc.next_id()}", ins=[], outs=[]
                            )
                            ev.engine = ins.engine
                            ev.sync_info = bass_rust.SyncInfo(
                                on_wait=chunk, on_update=[]
                            )
                            nc.register_instruction(ev, overwrite=True)
                            new_list.append(ev)
                        ins.sync_info = bass_rust.SyncInfo(
                            on_wait=keep,
                            on_update=list(si.on_update) if si.on_update else [],
                        )
                    new_list.append(ins)
                if changed:
                    insts[:] = new_list

    def _drain_and_barrier(self, tick_clock, wait_clock):
        self._split_excess_waits()
        ticks = list(tick_clock.global_clock)
        for p, t in enumerate(ticks):
            if t <= 0:
                continue
            v = bass_rust.VectorClock()
            v.require_at_least(p, t)
            d = self.nc.sync.drain()
            wait_clock.add_sem_waits(d.ins, ScopedClock({None: v}))
        self.nc.all_engine_barrier()
        popped = self.nc._tile_sem_poison_stack.pop()
        assert popped is self._sem_poison
        self.nc.clear_and_free_semaphores(list(self.sems.allocated().values()))
        self.nc.all_engine_barrier()


def build_nc() -> bass.Bass:
    nc = bass.Bass()

    # host-pretiled inputs
    xt_d = nc.dram_tensor("xt", [128, DT, S], BF16, kind="ExternalInput")
    wqkv_d = nc.dram_tensor("wqkv", [128, DT, 6 * 128], BF16, kind="ExternalInput")
    wo_d = nc.dram_tensor("wo", [128, NPAIR, D], BF16, kind="ExternalInput")
    bq_d = nc.dram_tensor("bq", [128, NPAIR], F32, kind="ExternalInput")
    out_d = nc.dram_tensor("out", [S, D], BF16, kind="ExternalOutput")

    with _TileContextSplitDrain(nc) as tc:
        with (
            tc.tile_pool(name="const", bufs=1) as constp,
            tc.tile_pool(name="pers", bufs=1) as pers,
            tc.tile_pool(name="dramsc", bufs=1, space="DRAM") as drp,
        ):
            bq_sb = constp.tile([128, NPAIR], F32, tag="bq")
            nc.sync.dma_start(out=bq_sb, in_=bq_d[:, :])
            wo_sb = constp.tile([128, NPAIR, D], BF16, tag="wo")
            nc.sync.dma_start(out=wo_sb, in_=wo_d[:, :, :])
            # per-d-tile DMAs so the first QKV matmuls start ~3us in, not
            # after the whole 5.5MB input load
            xT = pers.tile([128, DT, S], BF16, tag="xT")
            wq_sb = pers.tile([128, DT, 6 * 128], BF16, tag="wqkv")
            for d in range(DT):
                nc.sync.dma_start(out=wq_sb[:, d, :], in_=wqkv_d[:, d, :])
                nc.sync.dma_start(out=xT[:, d, :], in_=xt_d[:, d, :])

            QT = pers.tile([128, NPAIR, S], BF16, tag="QT")
            KT = pers.tile([128, NPAIR, S], BF16, tag="KT")
            ctxT = pers.tile([128, NPAIR, S], BF16, tag="ctxT")
            ctxR = pers.tile([128, NPAIR, S], BF16, tag="ctxR")
            V_aug = pers.tile([128, ST, HL, 66], BF16, tag="V_aug")
            den_d = drp.tile([2 * NSC, 2 * SC], F32, tag="den_d")
            recip_d = drp.tile([2 * NSC, 16, 64], F32, tag="recip_d")

            # ---------------- Phase 1: Q^T / K^T projections -----------------
            # 2-bank half-groups with 4-deep rotation keep the PE stream
            # dense across the evac boundaries; evacs alternate engines
            with tc.tile_pool(name="qkps", bufs=4, space="PSUM") as qkp:
                for gi, (proj, pair, ch) in enumerate(
                    (p, q, h) for p in range(2) for q in range(NPAIR)
                    for h in range(2)
                ):
                    ps = qkp.tile([128, 2, 512], F32, tag="qkps")
                    col = (proj * 2 + pair) * 128
                    for d in range(DT):
                        lhs = wq_sb[:, d, col:col + 128]
                        for c2 in range(2):
                            c4 = 2 * ch + c2
                            nc.tensor.matmul(
                                ps[:, c2, :],
                                lhs,
                                xT[:, d, c4 * 512:(c4 + 1) * 512],
                                start=(d == 0),
                                stop=(d == DT - 1),
                            )
                    for c2 in range(2):
                        c4 = 2 * ch + c2
                        dst = (QT if proj == 0 else KT)[
                            :, pair, c4 * 512:(c4 + 1) * 512
                        ]
                        if proj == 0:
                            if (gi + c2) % 2 == 0:
                                nc.vector.tensor_scalar_add(
                                    dst, ps[:, c2, :], bq_sb[:, pair:pair + 1]
                                )
                            else:
                                nc.scalar.activation(
                                    dst, ps[:, c2, :], AF.Identity,
                                    bias=bq_sb[:, pair:pair + 1],
                                )
                        else:
                            if (gi + c2) % 2 == 0:
                                nc.scalar.activation(dst, ps[:, c2, :], AF.Copy)
                            else:
                                nc.vector.tensor_copy(dst, ps[:, c2, :])

            # ---------------- Phase 2: V (direct layout) + ones column -------
            nc.vector.memset(V_aug[:, :, :, 64:65], 1.0)
            nc.vector.memset(V_aug[:, :, :, 65:66], 0.0)
            with tc.tile_pool(name="vps", bufs=6, space="PSUM") as vp:
                for t in range(ST):
                    ps = vp.tile([128, HL * DV], F32, tag="vps")
                    for d in range(DT):
                        nc.tensor.matmul(
                            ps,
                            xT[:, d, t * 128:(t + 1) * 128],
                            wq_sb[:, d, 512:768],
                            start=(d == 0),
                            stop=(d == DT - 1),
                        )
                    nc.scalar.activation(
                        V_aug[:, t, :, 0:64],
                        ps.rearrange("p (h v) -> p h v", h=HL),
                        AF.Copy,
                    )

            # ---------------- Phase 3: attention -----------------------------
            with (
                tc.tile_pool(name="ptp", bufs=6) as ptp,
                tc.tile_pool(name="repp", bufs=4) as repp,
                tc.tile_pool(name="denp", bufs=2) as denp,
                tc.tile_pool(name="outp", bufs=8) as outp,
            ):
                def norm_pair(c, pair, den_g):
                    """Normalize this pair's chunk: 1/den via a tall-skinny
                    [16,64] reshape (DMA reshuffle makes the 8-cyc/elem DVE
                    reciprocal ~0.55us), broadcast, scale."""
                    rp = 2 * c + pair
                    den_sb = denp.tile([16, 64], F32, name="densb", tag="densb")
                    recip_sb = denp.tile([16, 64], F32, name="recsb", tag="recsb")
                    nc.sync.dma_start(out=den_d[rp:rp + 1, :], in_=den_g)
                    nc.sync.dma_start(
                        out=den_sb,
                        in_=den_d[rp, :].unsqueeze(0)
                        .rearrange("q (p k) -> (q p) k", p=16),
                    )
                    nc.vector.reciprocal(recip_sb, den_sb)
                    nc.sync.dma_start(out=recip_d[rp, :, :], in_=recip_sb)
                    rep = repp.tile([128, SC], F32, tag="rep")
                    for e in range(2):
                        nc.sync.dma_start(
                            out=rep[64 * e:64 * (e + 1), :],
                            in_=recip_d[rp, 8 * e:8 * (e + 1), :]
                            .rearrange("p k -> (p k)").unsqueeze(0)
                            .to_broadcast((64, SC)),
                        )
                    nc.vector.tensor_mul(
                        ctxT[:, pair, c * SC:(c + 1) * SC],
                        ctxR[:, pair, c * SC:(c + 1) * SC],
                        rep,
                    )

                def outproj_tile(i, ops_pool):
                    ps = ops_pool.tile([128, 2, 512], F32, name="ops", tag="ops")
                    for dc in range(2):
                        for pair in range(NPAIR):
                            nc.tensor.matmul(
                                ps[:, dc, :],
                                ctxT[:, pair, i * 128:(i + 1) * 128],
                                wo_sb[:, pair, dc * 512:(dc + 1) * 512],
                                start=(pair == 0),
                                stop=(pair == NPAIR - 1),
                            )
                    # whole-tile evac on one engine, alternating per tile:
                    # halves the cross-engine handoffs in the drain
                    ot = outp.tile([128, D], BF16, tag="ot")
                    if i % 2 == 0:
                        nc.scalar.activation(ot, ps.rearrange("p a b -> p (a b)"),
                                             AF.Copy)
                    else:
                        nc.vector.tensor_copy(ot, ps.rearrange("p a b -> p (a b)"))
                    nc.sync.dma_start(
                        out=out_d[i * 128:(i + 1) * 128, :], in_=ot
                    )

                def attn_pair(c, pair, spp, cpp, extra=None):
                    cps = [
                        cpp.tile([65, SC], F32, name=f"cp{e}", tag="cp")
                        for e in range(2)
                    ]
                    for j in range(ST):
                        sps = [
                            spp.tile([128, SC], F32, name=f"sp{e}", tag="sp")
                            for e in range(2)
                        ]
                        for e in range(2):
                            nc.tensor.matmul(
                                sps[e],
                                KT[64 * e:64 * (e + 1), pair,
                                   j * 128:(j + 1) * 128],
                                QT[64 * e:64 * (e + 1), pair,
                                   c * SC:(c + 1) * SC],
                                start=True,
                                stop=True,
                            )
                        # exp alternates engines per j (exact LUT on ScalarE,
                        # Schraudolph bf16-bits on VectorE), emitted as two
                        # per-e halves so ctx_e0 unblocks after half the work
                        pt = ptp.tile([128, 2, SC], BF16, tag="pt")
                        if j % 2 == 0 or j == 15:
                            for e in range(2):
                                nc.scalar.activation(
                                    pt[:, e, :], sps[e], AF.Exp
                                )
                        else:
                            for e in range(2):
                                nc.vector.tensor_scalar(
                                    pt[:, e, :].bitcast(I16),
                                    sps[e],
                                    EXP_A,
                                    EXP_B,
                                    ALU.mult,
                                    ALU.add,
                                )
                        for e in range(2):
                            nc.tensor.matmul(
                                cps[e][0:65, :],
                                V_aug[:, j, 2 * pair + e, 0:65],
                                pt[:, e, :],
                                start=(j == 0),
                                stop=(j == ST - 1),
                            )
                        if extra is not None:
                            extra(pair * ST + j)
                    # stage raw ctx + denominators, then normalize
                    den_g = denp.tile([1, 2 * SC], F32, name="den", tag="den")
                    for e in range(2):
                        nc.vector.tensor_copy(
                            den_g[0:1, e * SC:(e + 1) * SC], cps[e][64:65, :]
                        )
                    nc.scalar.activation(
                        ctxR[0:64, pair, c * SC:(c + 1) * SC],
                        cps[0][0:64, :],
                        AF.Copy,
                    )
                    nc.vector.tensor_copy(
                        ctxR[64:128, pair, c * SC:(c + 1) * SC],
                        cps[1][0:64, :],
                    )
                    norm_pair(c, pair, den_g)

                with (
                    tc.tile_pool(name="spp", bufs=5, space="PSUM") as spp,
                    tc.tile_pool(name="cpp", bufs=3, space="PSUM") as cpp,
                ):
                    for c in range(NSC):
                        for pair in range(NPAIR):
                            attn_pair(c, pair, spp, cpp)

                # ------------ Phase 4: output projection ----------------------
                with tc.tile_pool(name="ops2", bufs=4, space="PSUM") as ops2:
                    for i in range(ST):
                        outproj_tile(i, ops2)

    return nc


_NC_CACHE = None


def get_nc() -> bass.Bass:
    global _NC_CACHE
    if _NC_CACHE is None:
        _NC_CACHE = build_nc()
    return _NC_CACHE


def prep_in_maps(hidden_state, Wq, bq, Wk, bk, Wv, bv, Wo, bo):
    hidden_state = np.asarray(hidden_state, np.float32)
    Wq, bq = np.asarray(Wq, np.float32), np.asarray(bq, np.float32)
    Wk = np.asarray(Wk, np.float32)
    Wv = np.asarray(Wv, np.float32)
    Wo = np.asarray(Wo, np.float32)
    scale = np.float32(1.0 / np.sqrt(DK))

    # shared per-batch transposed activations: [128, DT, S] bf16
    xts = []
    for b in range(B):
        xt = np.ascontiguousarray(hidden_state[b].T)          # [D, S]
        xt = xt.reshape(DT, 128, S).transpose(1, 0, 2)        # [128, DT, S]
        xts.append(np.ascontiguousarray(xt.astype(NPBF16)))

    in_maps = []
    for core in range(N_CORES):
        b, g = core // 4, core % 4
        hs = slice(HL * g, HL * (g + 1))
        # [4, D, DK] heads -> pair-major column blocks of 128
        wq = (Wq[hs] * scale).transpose(1, 0, 2).reshape(D, HL * DK)
        wk = Wk[hs].transpose(1, 0, 2).reshape(D, HL * DK)
        wv = Wv[hs].transpose(1, 0, 2).reshape(D, HL * DV)    # head-major cols
        wqkv = np.concatenate([wq, wk, wv], axis=1)           # [D, 768]
        wqkv = wqkv.reshape(DT, 128, 6 * 128).transpose(1, 0, 2)
        wo = Wo[HL * DV * g: HL * DV * (g + 1)]               # [256, D]
        wo = wo.reshape(NPAIR, 128, D).transpose(1, 0, 2)     # [128, 2, D]
        bq_p = (bq[hs] * scale).reshape(NPAIR, 128).T         # [128, 2]
        in_maps.append({
            "xt": xts[b],
            "wqkv": np.ascontiguousarray(wqkv.astype(NPBF16)),
            "wo": np.ascontiguousarray(wo.astype(NPBF16)),
            "bq": np.ascontiguousarray(bq_p.astype(np.float32)),
        })
    return in_maps


def gather(results, bv, Wo, bo):
    """Sum the 4 row-parallel partials per batch + constant bias row."""
    bias = (
        np.asarray(bv, np.float32).reshape(H * DV) @ np.asarray(Wo, np.float32)
        + np.asarray(bo, np.float32)
    )
    out = np.empty((B, S, D), np.float32)
    for b in range(B):
        acc = results[4 * b]["out"].astype(np.float32)
        for g in range(1, 4):
            acc = acc + results[4 * b + g]["out"]
        out[b] = acc + bias
    return out


def kernel(**inputs) -> np.ndarray:
    nc = get_nc()
    in_maps = prep_in_maps(**inputs)
    res = run_bass_kernel_spmd(nc, in_maps, core_ids=list(range(N_CORES)))
    return gather(res.results, inputs["bv"], inputs["Wo"], inputs["bo"])
